# revision 41
# baseline (speedup 1.0000x reference)
"""Trainium2 Bass kernel for the DynamicMemory routing module.

Computation (see reference):
    cat = concat([M_emb, Ht_n], 1)                  # [B, T', K]   B=8, T'=320, K=64
    u   = einsum('itdk,btk->bitd', W, cat)          # [B, M, T', D]  M=64, D=64
    3x { b = einsum('bid,bitd->bit', m, u); alph = softmax(b, -1)
         s = tanh(einsum('bit,bitd->bid', alph, u)); m = squash(s) }

Sharding: memory-slot axis i (M=64) split across 8 cores (8 slots each); every
core runs the identical program on its W slice and batch-wide activations, and
the host concatenates the per-core [B, 8, D] outputs.  No collectives.

Per-core kernel:
 - W is stored in HBM as e3m4 (x128 host-side scale; the 1/128 compensation is
   folded into the bf16 cat stationary).  One resident SBUF tile holds the
   whole 10.5MB slice; 20 per-group HWDGE DMAs on the SP ring stream it while
   the ACT ring carries cat/masks/m0.  Stage-1 PE matmuls (bf16 cat
   stationary x fp8 moving W, 160 x 512 cols) are the phase-1 bound (~36us)
   with DMA (~33us) just underneath.  PSUM evictions all run on ACT.
 - Iteration 1 runs INSIDE phase 1: its m broadcast only needs m0, so the
   logits mult+tree (DVE, idle during phase 1) runs per 5-group chunk as u
   groups land, exp (ACT) and wdiag (Pool) trail, and the Z + alph-weighted
   PE matmuls are interleaved into the stage-1 matmul stream.
 - Iterations 2-3: chunked pipeline (4 chunks of t-groups): logits via DVE
   multiply + binary-tree d-reduction (bf16, 2x mode); exp on ACT; Z and
   s_raw via mask-stationary PE matmuls accumulating onto a memset PSUM bank;
   tanh -> bf16 s_t; next m_bc broadcast via a bf16 PE matmul.  The squash
   scale is deferred into the next iteration's logits (sc_prev); sqrt via
   bit-trick + Newton on the DVE so only one ACT table set loads.  The final
   iteration ships s (f32) and the host applies the last squash in fp64.
"""

import sys

import numpy as np

try:
    import concourse.bacc as bacc
    import concourse.tile as tile
    from concourse import mybir
    from concourse.bass_utils import run_bass_kernel_spmd
except ImportError:
    sys.path.insert(0, "/opt/trn_rl_repo")
    import concourse.bacc as bacc
    import concourse.tile as tile
    from concourse import mybir
    from concourse.bass_utils import run_bass_kernel_spmd

F32 = mybir.dt.float32
BF16 = mybir.dt.bfloat16
FP8 = mybir.dt.float8e3
AF = mybir.ActivationFunctionType
ALU = mybir.AluOpType

B, MSLOT, T, D, K = 8, 64, 256, 64, 64
TT = MSLOT + T            # 320 routing targets
NCORES = 8
IL = MSLOT // NCORES      # 8 slots per core
G = TT // 16              # 20 groups of 16 t-values (one PSUM tile each)
NMM = G * 8               # 160 stage-1 matmuls (4 strips x 2 k-halves per group)
GW = 8 * IL * D           # 4096 fp8 W elements per group per partition
EPS = 1e-4
N_ITERS = 3
WSCALE = 128.0            # host-side W scale (e3m4 max 15.5; |W|max*128 = 6.9)

# u partition layout: p = 32*q + 8*t4 + b, with t = 16*g + 4*q + t4.
# Stage-1 matmul for (g, q, eta): K=(t4, k32)=128 block-diagonal over t4,
# M=32=(t4, b), accumulating the two k-halves eta in PSUM. Output strip is
# 32-aligned (hardware requires engine partition bases to be 0 mod 32).

_BF16_NP = mybir.dt.np(BF16)
_FP8_NP = mybir.dt.np(FP8)

_CHUNKS = [0, 7, 13, 19, 20]     # iters 2-3 pipeline chunks (1-group tail)
_CH1 = [0, 5, 10, 15, 19, 20]    # iter-1 chunks (small last chunk: short tail)
_CH1_2A = {4: 0, 9: 1, 14: 2, 18: 3, 19: 4}   # group -> 2a chunk to emit after
# exp+wdg two groups later: an exp emitted right after its chunk's last
# eviction would sit ahead of later evictions in the in-order ACT queue and
# stall them (PSUM backpressure -> PE stalls).
_CH1_EXP = {6: 0, 11: 1, 16: 2}
_CH1_PE = {8: 0, 13: 1, 18: 2}                # group -> PE (Z+2b) chunk hook


def _build_program(n_iters=N_ITERS, do_2b=True, do_2a=True):
    nc = bacc.Bacc("TRN2", target_bir_lowering=False, debug=False, num_devices=NCORES)

    wprep = nc.declare_dram_parameter("wprep", [128, G * GW], FP8, isOutput=False)
    catk = nc.declare_dram_parameter("catk", [128, NMM * 32], BF16, isOutput=False)
    m0 = nc.declare_dram_parameter("m0", [B, IL * D], BF16, isOutput=False)
    bmask = nc.declare_dram_parameter("bmask", [128, B], BF16, isOutput=False)
    bcmask = nc.declare_dram_parameter("bcmask", [B, 128], BF16, isOutput=False)
    mout = nc.declare_dram_parameter("mout", [B, IL * D], F32, isOutput=True)

    with tile.TileContext(nc) as tc:
        with (
            tc.tile_pool(name="const", bufs=1) as const,
            tc.tile_pool(name="upool", bufs=1) as upool,
            tc.tile_pool(name="work", bufs=2) as work,
            # 2a tiles live in a single-buffer pool: the WAR chain through
            # these buffers forces the Tile scheduler to software-pipeline
            # the chunks (mult/tree of chunk k+1 interleaves right behind
            # chunk k's consumers instead of all trees batching up).
            tc.tile_pool(name="tpool", bufs=1) as tpool,
            tc.tile_pool(name="ppmb", bufs=1, space="PSUM") as pool_pmb,
            tc.tile_pool(name="ppz", bufs=1, space="PSUM") as pool_pz,
            tc.tile_pool(name="pps", bufs=2, space="PSUM") as pool_ps,
        ):
            # whole W slice resident in SBUF (10.5MB fp8)
            w_all = const.tile([128, G, 8, IL * D], FP8)

            def emit_w(g):
                nc.sync.dma_start(
                    out=w_all[:, g, :, :],
                    in_=wprep[:, g * GW : (g + 1) * GW].rearrange(
                        "p (e f) -> p e f", e=8
                    ),
                )

            # tiny aux inputs first (the iteration-1 broadcast needs them
            # within ~2us), then cat part 1 (first matmuls + PE warmups),
            # then the W stream; cat part 2 trails on the ACT ring.
            bmask_sb = const.tile([128, B], BF16)
            nc.scalar.dma_start(out=bmask_sb, in_=bmask[:])
            bcmask_sb = const.tile([B, 128], BF16)
            nc.scalar.dma_start(out=bcmask_sb, in_=bcmask[:])
            m_first = const.tile([B, IL * D], BF16)
            nc.scalar.dma_start(out=m_first, in_=m0[:])
            cat_sb = const.tile([128, NMM * 32], BF16)
            nc.sync.dma_start(out=cat_sb[:, 0:1024], in_=catk[:, 0:1024])
            emit_w(0)
            emit_w(1)
            nc.scalar.dma_start(out=cat_sb[:, 1024:], in_=catk[:, 1024:])

            # tiny constant tiles for the GPSIMD scale chain (it only
            # accepts tensor-tensor shaped ops, no immediates)
            chalf = const.tile([B, IL], F32)
            nc.gpsimd.memset(chalf[:], 0.5)
            ceps = const.tile([B, IL], F32)
            nc.gpsimd.memset(ceps[:], EPS)
            cone = const.tile([B, IL], F32)
            nc.gpsimd.memset(cone[:], 1.0)
            cneg1 = const.tile([B, IL], F32)
            nc.gpsimd.memset(cneg1[:], -1.0)

            # u[p=(q,t4,b), (g, i, d)] in bf16.
            u = upool.tile([128, G, IL, D], BF16)

            # PE warmup: the tensor engine ramps 0.65 -> 1.2 -> 2.4 GHz over
            # ~3us of continuous execution; a dozen throwaway matmuls on the
            # just-landed cat tile finish the ramp before the real stage-1
            # stream starts, and chain into it with no idle gap.
            warm = pool_ps.tile([B, IL * D], F32, tag="ps")
            for _ in range(12):
                nc.tensor.matmul(warm[:], lhsT=cat_sb[:, 0:8],
                                 rhs=cat_sb[:, 0:512], start=True, stop=True)

            # iteration-1 broadcast: m_bc[p, (i,d)] = m0[b(p), .] -- its
            # inputs are the first DMAs, so this runs at ~1.5us.
            pmb = pool_pmb.tile([128, IL * D], F32, tag="pmb")
            nc.tensor.matmul(pmb[:], lhsT=bcmask_sb[:], rhs=m_first[:],
                             start=True, stop=True)
            m_bc = work.tile([128, IL, D], BF16, tag="mbc")
            nc.scalar.copy(out=m_bc,
                           in_=pmb[:].rearrange("p (i d) -> p i d", i=IL))

            def chunk_2a(g0, g1, m_bc, sc_prev, wexp, wdg, wdg_eng):
                logits = chunk_2a_core(g0, g1, m_bc, sc_prev)
                chunk_expwdg(g0, g1, logits, wexp, wdg, wdg_eng)

            def chunk_expwdg(g0, g1, logits, wexp, wdg, wdg_eng):
                CG = g1 - g0
                gs = slice(g0, g1)
                # w = exp(logits); |logits| is small, no max-sub needed
                nc.scalar.activation(wexp[:, gs, :], logits, AF.Exp)
                wdg_eng.tensor_mul(
                    wdg[:, gs, :, :],
                    bmask_sb[:, None, None, :].broadcast_to([128, CG, IL, B]),
                    wexp[:, gs, :, None].broadcast_to([128, CG, IL, B]),
                )

            def chunk_2a_core(g0, g1, m_bc, sc_prev):
                """logits mult + tree for groups [g0, g1); returns logits AP."""
                CG = g1 - g0
                MAXCG = 7
                gs = slice(g0, g1)
                ve = nc.vector
                tmp_f = tpool.tile([128, MAXCG, IL, D], BF16, tag="tmp")
                tmp = tmp_f[:, 0:CG]
                if do_2a:
                    ve.tensor_mul(
                        tmp,
                        u[:, gs, :, :],
                        m_bc[:, None, :, :].broadcast_to([128, CG, IL, D]),
                    )
                else:
                    ve.memset(tmp.rearrange("p g i d -> p (g i d)"), 0.5)
                cur = tmp
                for w_ in (32, 16, 8, 4, 2):
                    nxt_f = tpool.tile([128, MAXCG, IL, w_], BF16, tag=f"r{w_}")
                    nxt = nxt_f[:, 0:CG]
                    ve.tensor_add(
                        nxt, cur[:, :, :, 0:w_], cur[:, :, :, w_ : 2 * w_]
                    )
                    cur = nxt
                logits_f = tpool.tile([128, MAXCG, IL], F32, tag="lg")
                logits = logits_f[:, 0:CG]
                ve.tensor_add(
                    logits[:, :, :, None], cur[:, :, :, 0:1], cur[:, :, :, 1:2]
                )
                if sc_prev is not None:
                    # m_bc was the unscaled tanh output; the squash scale
                    # factors out of the d-contraction and is applied to the
                    # logits instead.
                    lgs_f = tpool.tile([128, MAXCG, IL], F32, tag="lgs")
                    lgs = lgs_f[:, 0:CG]
                    nc.vector.tensor_mul(
                        lgs,
                        logits,
                        sc_prev[:, None, :].broadcast_to([128, CG, IL]),
                    )
                    logits = lgs
                return logits

            def chunk_pe(g0, g1, wexp, wdg, ps, pz, zstop):
                nc.tensor.matmul(
                    pz[:, g0 * IL : g1 * IL],
                    lhsT=bmask_sb[:],
                    rhs=wexp[:, g0:g1, :].rearrange("p g i -> p (g i)"),
                    start=False,
                    stop=zstop,
                )
                if do_2b:
                    for j in range(g0, g1):
                        for i in range(IL):
                            nc.tensor.matmul(
                                ps[:, i * D : (i + 1) * D],
                                lhsT=wdg[:, j, i, :],
                                rhs=u[:, j, i, :],
                                start=False,
                                stop=(j == G - 1),
                            )

            def iter_tail(it, last_it, ps, pz):
                """softmax normalize + tanh (+ squash prep, next m_bc).

                Emitted at high scheduler priority: the sqrt/scale Newton
                chain is ~14 tiny serial DVE ops, and at default priority the
                Tile list scheduler interleaves them between the NEXT
                iteration's big tree ops, stretching the chain (and the scB
                scale everything downstream waits on) across ~9us.
                """
                with tc.high_priority():
                    return _iter_tail(it, last_it, ps, pz)

            def _iter_tail(it, last_it, ps, pz):
                zz = work.tile([B, IL], F32, tag="zz")
                nc.vector.tensor_reduce(
                    out=zz,
                    in_=pz[:].rearrange("b (g i) -> b i g", g=G),
                    axis=mybir.AxisListType.X,
                    op=ALU.add,
                )
                rz = work.tile([B, IL], F32, tag="rz")
                nc.vector.reciprocal(rz, zz[:])
                # fused s = tanh(ps * rz): per-slot ACT ops with a
                # per-partition AP scale read ps straight out of PSUM --
                # drops the DVE normalize multiply and one engine hop from
                # the serial tail.
                s_t = work.tile([B, IL, D], F32 if last_it else BF16, tag="st")
                psv = ps[:].rearrange("b (i d) -> b i d", i=IL)
                for i in range(IL):
                    nc.scalar.activation(s_t[:, i, :], psv[:, i, :], AF.Tanh,
                                         scale=rz[:, i : i + 1])
                s_tf = s_t[:].rearrange("b i d -> b (i d)")
                if last_it:
                    nc.sync.dma_start(out=mout[:], in_=s_tf)
                    return None, None

                # squash: q = sum_d s^2 ; n = sqrt(q) + EPS ; scale = n/(1+n^2)
                sq = work.tile([B, IL * D], BF16, tag="sq")
                nc.vector.tensor_mul(sq, s_tf, s_tf)
                q = work.tile([B, IL], F32, tag="q")
                nc.vector.tensor_reduce(
                    out=q,
                    in_=sq[:].rearrange("b (i d) -> b i d", i=IL),
                    axis=mybir.AxisListType.X,
                    op=ALU.add,
                )
                # The scale chain runs on the (otherwise idle) GPSIMD engine:
                # the ~15 tiny serial ops would otherwise interleave between
                # the next iteration's big DVE tree ops and stretch across
                # ~9us, blocking the scB scale every chunk's exp waits on.
                # GPSIMD only accepts TensorTensor-shaped ops, so scalars
                # come from small constant tiles; pow/divide run in the Q7
                # software ALU.
                PL = nc.gpsimd
                nn = work.tile([B, IL], F32, tag="nn")
                PL.tensor_tensor(out=nn, in0=q[:], in1=chalf[:], op=ALU.pow)
                nne = work.tile([B, IL], F32, tag="nne")
                PL.tensor_tensor(out=nne, in0=nn[:], in1=ceps[:], op=ALU.add)
                n2 = work.tile([B, IL], F32, tag="n2")
                PL.tensor_mul(n2, nne[:], nne[:])
                d1 = work.tile([B, IL], F32, tag="d1")
                PL.tensor_tensor(out=d1, in0=n2[:], in1=cone[:], op=ALU.add)
                rd1 = work.tile([B, IL], F32, tag="rd1")
                PL.tensor_tensor(out=rd1, in0=d1[:], in1=cneg1[:], op=ALU.pow)
                sc = work.tile([B, IL], F32, tag="sc")
                PL.tensor_mul(sc, nne[:], rd1[:])

                # next iteration's m_bc = broadcast of the UNSCALED tanh
                # output; bf16 s_t keeps the PE broadcast at 1 cyc/row.
                pmb2 = pool_pmb.tile([128, IL * D], F32, tag="pmb")
                nc.tensor.matmul(pmb2[:], lhsT=bcmask_sb[:], rhs=s_tf,
                                 start=True, stop=True)
                nm_bc = work.tile([128, IL, D], BF16, tag="mbc")
                nc.scalar.copy(
                    out=nm_bc, in_=pmb2[:].rearrange("p (i d) -> p i d", i=IL)
                )
                sc_bf = work.tile([B, IL], BF16, tag="scbf")
                nc.gpsimd.tensor_copy(out=sc_bf, in_=sc[:])
                pscb = pool_pz.tile([128, IL], F32, tag="pz")
                nc.tensor.matmul(pscb[:], lhsT=bcmask_sb[:], rhs=sc_bf[:],
                                 start=True, stop=True)
                scB = work.tile([128, IL], F32, tag="scB")
                nc.scalar.copy(out=scB, in_=pscb[:])
                return nm_bc, scB

            # ---- stage 1 + iteration 1 (interleaved) ----
            wexp1 = work.tile([128, G, IL], BF16, tag="wexp")
            wdg1 = work.tile([128, G, IL, B], BF16, tag="wdg")
            ps1 = pool_ps.tile([B, IL * D], F32, tag="ps")
            nc.scalar.memzero(ps1[:])
            pz1 = pool_pz.tile([B, G * IL], F32, tag="pz")
            nc.scalar.memzero(pz1[:])

            # iter-1 chunk c: 2a emitted after its last group's eviction; its
            # PE ops (Z + 2b) interleave into the stage-1 matmul stream a
            # couple of groups later (so the PE never stalls waiting on wdg).
            NCH1 = len(_CH1) - 1
            lg1 = {}
            with tc.tile_pool(name="psum_u", bufs=4, space="PSUM") as psum_u:
                for g in range(G):
                    if g + 2 < G:
                        emit_w(g + 2)
                    if g in _CH1_PE and n_iters > 0:
                        c = _CH1_PE[g]
                        chunk_pe(_CH1[c], _CH1[c + 1], wexp1, wdg1, ps1, pz1,
                                 zstop=False)
                    pg = psum_u.tile([128, IL * D], F32, tag="pu")
                    for q in range(4):
                        for eta in range(2):
                            idx = (g * 4 + q) * 2 + eta
                            nc.tensor.matmul(
                                pg[32 * q : 32 * (q + 1), :],
                                lhsT=cat_sb[:, idx * 32 : (idx + 1) * 32],
                                rhs=w_all[:, g, q * 2 + eta, :],
                                start=(eta == 0),
                                stop=(eta == 1),
                                tile_position=(0, 32 * q),
                            )
                    nc.scalar.copy(
                        out=u[:, g, :, :],
                        in_=pg[:].rearrange("p (i d) -> p i d", i=IL),
                    )
                    if g in _CH1_EXP and n_iters > 0:
                        c = _CH1_EXP[g]
                        chunk_expwdg(_CH1[c], _CH1[c + 1], lg1[c], wexp1, wdg1,
                                     nc.gpsimd)
                    if g in _CH1_2A and n_iters > 0:
                        c = _CH1_2A[g]
                        lg1[c] = chunk_2a_core(_CH1[c], _CH1[c + 1], m_bc, None)

                if n_iters > 0:
                    chunk_expwdg(_CH1[3], _CH1[4], lg1[3], wexp1, wdg1,
                                 nc.gpsimd)
                    chunk_expwdg(_CH1[4], _CH1[5], lg1[4], wexp1, wdg1,
                                 nc.vector)
                    chunk_pe(_CH1[3], _CH1[4], wexp1, wdg1, ps1, pz1,
                             zstop=False)
                    chunk_pe(_CH1[4], _CH1[5], wexp1, wdg1, ps1, pz1,
                             zstop=True)
                    m_bc, sc_prev = iter_tail(0, n_iters == 1, ps1, pz1)

                # ---- iterations 2..n ----
                for it in range(1, n_iters):
                    last_it = it == n_iters - 1
                    wexp = work.tile([128, G, IL], BF16, tag="wexp")
                    wdg = work.tile([128, G, IL, B], BF16, tag="wdg")
                    ps = pool_ps.tile([B, IL * D], F32, tag="ps")
                    nc.scalar.memzero(ps[:])
                    pz = pool_pz.tile([B, G * IL], F32, tag="pz")
                    nc.scalar.memzero(pz[:])
                    NCH = len(_CHUNKS) - 1
                    for ch in range(NCH):
                        g0, g1 = _CHUNKS[ch], _CHUNKS[ch + 1]
                        chunk_2a(g0, g1, m_bc, sc_prev, wexp, wdg,
                                 nc.vector if ch == NCH - 1 else nc.gpsimd)
                        chunk_pe(g0, g1, wexp, wdg, ps, pz,
                                 zstop=(ch == NCH - 1))
                    m_bc, sc_prev = iter_tail(it, last_it, ps, pz)

                if n_iters == 0:
                    nc.gpsimd.dma_start(out=mout[:], in_=m_first)

    nc.compile()
    return nc


_NC_CACHE = None


def _get_program():
    global _NC_CACHE
    if _NC_CACHE is None:
        _NC_CACHE = _build_program()
    return _NC_CACHE


def _host_prep(M_emb, Ht_n, new_M_emb_init, W):
    """Build per-core input maps."""
    cat = np.concatenate([M_emb, Ht_n], axis=1).astype(np.float32)  # [B, TT, K]
    cat = cat * (1.0 / WSCALE)  # compensate the fp8 W scale

    # catk[(t4,k32), ((g,q,eta), (t4',b))] = cat[b, 16g+4q+t4', 32*eta+k32]
    # on the t4==t4' diagonal blocks, else 0.
    catr = cat.transpose(1, 2, 0).reshape(G, 4, 4, 2, 32, B)  # [g,q,t4,eta,k32,b]
    catbd = np.zeros((4, 32, G, 4, 2, 4, B), np.float32)      # [t4,k32,g,q,eta,t4',b]
    for t4 in range(4):
        catbd[t4, :, :, :, :, t4, :] = catr[:, :, t4, :, :, :].transpose(3, 0, 1, 2, 4)
    catk = catbd.reshape(128, NMM * 32).astype(_BF16_NP)

    # W [i, t, d, k] -> per-core wprep[(t4,k32), (g, q, eta, i_l, d)] fp8-e3m4
    # with t = 16g + 4q + t4, k = 32*eta + k32, scaled by WSCALE
    Wt = np.ascontiguousarray(W.transpose(1, 3, 0, 2))  # [t, k, i, d]
    Wr = Wt.reshape(G, 4, 4, 2, 32, MSLOT, D)           # [g, q, t4, eta, k32, i, d]
    Wr = Wr.transpose(2, 4, 0, 1, 3, 5, 6)              # [t4, k32, g, q, eta, i, d]
    Wr = Wr * WSCALE

    bmask = np.zeros((128, B), np.float32)
    for p in range(128):
        bmask[p, p % B] = 1.0
    bcmask = np.ascontiguousarray(bmask.T)

    in_maps = []
    for c in range(NCORES):
        wc = Wr[:, :, :, :, :, c * IL : (c + 1) * IL, :]
        wc = np.ascontiguousarray(wc).reshape(128, G * GW).astype(_FP8_NP)
        m0c = (
            new_M_emb_init[:, c * IL : (c + 1) * IL, :]
            .reshape(B, IL * D)
            .astype(_BF16_NP)
        )
        in_maps.append(
            {
                "wprep": wc,
                "catk": catk,
                "m0": m0c,
                "bmask": bmask.astype(_BF16_NP),
                "bcmask": bcmask.astype(_BF16_NP),
            }
        )
    return in_maps


def run(inputs, trace=False, **kwargs):
    """Run on hardware; returns (full_output [B, M, D] f32, BassKernelResults)."""
    nc = _get_program()
    in_maps = _host_prep(
        np.asarray(inputs["M_emb"], np.float32),
        np.asarray(inputs["Ht_n"], np.float32),
        np.asarray(inputs["new_M_emb_init"], np.float32),
        np.asarray(inputs["W"], np.float32),
    )
    res = run_bass_kernel_spmd(
        nc, in_maps, core_ids=list(range(NCORES)), trace=trace, **kwargs
    )
    # the device ships the final tanh output s; the last squash runs here in
    # fp64
    parts = []
    for c in range(NCORES):
        s = np.asarray(res.results[c]["mout"], np.float64).reshape(B, IL, D)
        q = (s * s).sum(axis=-1)
        n = np.sqrt(q) + EPS
        parts.append(s * (n / (1.0 + n * n))[:, :, None])
    full = np.concatenate(parts, axis=1).astype(np.float32)  # [B, M, D]
    return full, res


def kernel(**inputs) -> np.ndarray:
    out, _ = run(inputs, trace=False)
    return out
